# revision 71
# baseline (speedup 1.0000x reference)
"""DecoderLSTM w/ Bahdanau attention for trn2, 8 NeuronCores, data-parallel over batch.

Sharding: batch B=32 -> 4 samples/core, zero collectives.
Host precomputes: enc_proj (+benc+bdec) transposed per a-chunk; emb-part of
LSTM gates (+biases) per step; gate reorder i,f,g,o -> i,f,o,g so
sigmoid(i,f,o) is one tanh(x/2) ACT instr; fc weights in [128, k*V+v] layout
for single-DMA n-chunk loads.
Phase 1: 99 sequential steps fully SBUF-resident:
  dec_proj.T [A-part,(a,b)] -> DVE per-partition-scalar adds (4x mode) into
  s_arg [128, (a,b,s)] -> 4 big ACT tanh [128,1600] (pipelines with scores)
  scores = v.T @ T (tile-position packed into psum rows {0,32,64,96})
  single exp+accum over [128,400]; alpha.T (raw exp) via 4 PE transposes;
  softmax 1/Z deferred: an off-chain matmul of broadcast(1/Z) against a
  selection matrix builds a per-column inverse tile applied at the ctx.T
  psum->sbuf copy. Fused context.T: 64 accumulating matvecs produce
  pcT[:, k*4+b] directly (no separate ctx + transpose stages).
  gates = h-part (early, fills the tanh window) + ctx-part weight-moving MMs;
  emb-part preloaded into psum via identity matmul, gate transposes
  accumulate onto it; activations read psum; sigmoid(i,f,o)+u1 fire after
  quadrant 2 while PE finishes g.
  Phase-2 logit tiles (h_hist @ fc_W.T, bf16 block-layout out, host
  reassembles) pumped into PE idle windows as their 32-step h block
  completes; ni-major remainder with prefetched weight loads and psum-bank
  rotation over the then-idle gate banks.
"""

import numpy as np
import ml_dtypes

import concourse.bass as bass
import concourse.bacc as bacc
import concourse.mybir as mybir
import concourse.tile as tile
from concourse.bass_utils import run_bass_kernel_spmd

B, S, T = 32, 400, 100
E, H, A, V = 256, 512, 512, 32000
G = 4 * H
NC = 8
BL = B // NC          # 4 local samples
TS = T - 1            # 99 steps
F32 = mybir.dt.float32
BF16 = mybir.dt.bfloat16
AF = mybir.ActivationFunctionType

HC = H // 128   # 4
ACk = A // 128  # 4
KC = 2 * HC     # 8 gate K chunks (ctx, h)
GC = G // 128   # 16
S_CHUNKS = [(i * 128, min(128, S - i * 128)) for i in range((S + 127) // 128)]
PH2_PER_STEP = 4


def ceil_div(a, b):
    return (a + b - 1) // b


DEBUG = False


def build_nc(n_steps=TS, with_fcb=False):
    nc = bacc.Bacc()
    ts_rows = n_steps * BL
    if DEBUG:
        d_dbg = {
            "dT": nc.dram_tensor("dbg_dT", [128, 16], F32, kind="ExternalOutput"),
            "alpha": nc.dram_tensor("dbg_alpha", [128, S], F32, kind="ExternalOutput"),
            "cT": nc.dram_tensor("dbg_cT", [128, 16], F32, kind="ExternalOutput"),
            "act": nc.dram_tensor("dbg_act", [128, 64], F32, kind="ExternalOutput"),
            "arg": nc.dram_tensor("dbg_arg", [128, 6400], F32, kind="ExternalOutput"),
        }

    d_enc = nc.dram_tensor("enc", [BL, S, H], BF16, kind="ExternalInput")
    d_encp = nc.dram_tensor("encp", [128, ACk * BL * S], BF16, kind="ExternalInput")
    d_h0T = nc.dram_tensor("h0T", [H, BL], BF16, kind="ExternalInput")
    d_c0t = nc.dram_tensor("c0t", [128, 4 * BL], F32, kind="ExternalInput")
    d_embb = nc.dram_tensor("embb", [128, n_steps * 4 * GC], BF16, kind="ExternalInput")
    d_WgT = nc.dram_tensor("WgT", [2 * H, G], BF16, kind="ExternalInput")
    d_Wdec = nc.dram_tensor("Wdec", [H, A], BF16, kind="ExternalInput")
    d_v = nc.dram_tensor("vt", [128, 4], BF16, kind="ExternalInput")
    d_ident = nc.dram_tensor("ident", [128, 128], F32, kind="ExternalInput")
    d_id16 = nc.dram_tensor("id16", [128, 128], BF16, kind="ExternalInput")
    # selection matrix: SEL[32b, k*4+b] = 1 -> invcols[p, k*4+b] = 1/Z_b
    d_sel = nc.dram_tensor("sel", [128, 4 * BL], F32, kind="ExternalInput")
    # fc weights: [128, k*V + v] so one n-chunk slice is 4 strided ranges
    # fetched in a single DMA
    d_fcW2 = nc.dram_tensor("fcW2", [128, HC * V], BF16, kind="ExternalInput")
    d_fcb = nc.dram_tensor("fcb", [1, V], BF16, kind="ExternalInput")

    n_chunks = [(i * 512, min(512, V - i * 512)) for i in range(ceil_div(V, 512))]
    m_chunks = [(i * 128, min(128, ts_rows - i * 128))
                for i in range(ceil_div(ts_rows, 128))]
    # block-layout output: one contiguous DMA per logit tile; host reassembles
    d_out = nc.dram_tensor(
        "out", [len(m_chunks), len(n_chunks), 128, 512], BF16,
        kind="ExternalOutput")

    with tile.TileContext(nc) as tc:
        import contextlib
        stack = contextlib.ExitStack()
        with stack:
            P = lambda name, bufs, space="SBUF": stack.enter_context(
                tc.tile_pool(name=name, bufs=bufs, space=space))
            singles = P("singles", 1)
            trans = P("trans", 4)    # transient sbuf (ph2 fw tiles)
            st = P("st", 2)          # small per-step sbuf scratch
            stg = P("stg", 1)        # [4, 2048] gates sbuf
            stc = P("stc", 2)        # c state ping-pong
            Tp = P("Tp", 1)          # big [128, 6400] tanh-arg tile
            ob = P("ob", 6)          # phase2 out staging
            # PSUM: exactly 8 banks
            p_small = P("p_small", 2, space="PSUM")   # tag psm, <=512B
            p_sccx = P("p_sccx", 1, space="PSUM")     # shared scores/ctx bank
            p_g = P("p_g", 1, space="PSUM")           # tags pg0..3, 2KB each
            p_ph2 = P("p_ph2", 1, space="PSUM")       # phase2 logits bank

            # ---- persistent SBUF ----
            sb_enc = [[singles.tile([sp, H], BF16, tag=f"enc_{b}_{ci}", name=f"enc_{b}_{ci}")
                       for ci, (so, sp) in enumerate(S_CHUNKS)] for b in range(BL)]
            for b in range(BL):
                for ci, (so, sp) in enumerate(S_CHUNKS):
                    nc.sync.dma_start(out=sb_enc[b][ci], in_=d_enc[b, so:so + sp, :])
            sb_encp = singles.tile([128, ACk * BL * S], BF16, tag="encp", name="encp")
            nc.sync.dma_start(out=sb_encp, in_=d_encp[:, :])
            sb_embb = singles.tile([128, n_steps * 4 * GC], BF16, tag="embb", name="embb")
            nc.sync.dma_start(out=sb_embb, in_=d_embb[:, :])
            sb_WgT = [singles.tile([128, G], BF16, tag=f"wg_{k}", name=f"wg_{k}") for k in range(KC)]
            for k in range(KC):
                nc.sync.dma_start(out=sb_WgT[k], in_=d_WgT[k * 128:(k + 1) * 128, :])
            sb_Wdec = [singles.tile([128, A], BF16, tag=f"wd_{k}", name=f"wd_{k}") for k in range(HC)]
            for k in range(HC):
                nc.sync.dma_start(out=sb_Wdec[k], in_=d_Wdec[k * 128:(k + 1) * 128, :])
            sb_v = singles.tile([128, 4], BF16)
            nc.sync.dma_start(out=sb_v, in_=d_v[:, :])
            sb_id = singles.tile([128, 128], F32)
            nc.sync.dma_start(out=sb_id, in_=d_ident[:, :])
            sb_id16 = singles.tile([128, 128], BF16)
            nc.sync.dma_start(out=sb_id16, in_=d_id16[:, :])
            sb_sel = singles.tile([128, 4 * BL], F32)
            nc.sync.dma_start(out=sb_sel, in_=d_sel[:, :])
            # h history: one tile, col = k*HB + t*BL + b (HB per k-chunk)
            HB = (TS + 1) * BL
            sb_hh = singles.tile([128, HC * HB], BF16, tag="hh", name="hh")
            sb_h = [sb_hh[:, k * HB:(k + 1) * HB] for k in range(HC)]
            for k in range(HC):
                nc.sync.dma_start(out=sb_hh[:, k * HB:k * HB + BL],
                                  in_=d_h0T[k * 128:(k + 1) * 128, :])
            if with_fcb:
                ones_sb = singles.tile([1, 128], BF16)
                nc.vector.memset(ones_sb, 1.0)
                fcb_sb = singles.tile([1, V], BF16)
                nc.sync.dma_start(out=fcb_sb, in_=d_fcb[:, :])

            # tanh-argument tile, written by DVE scalar-adds, tanh in-place
            s_arg = Tp.tile([128, ACk * BL * S], BF16, tag="targ", name="targ")

            # persistent psum bank: scores in cols 0:400 (rows 32b), gates.T
            # (+emb preload) in cols 448:512. Zero-init via matmul once so the
            # single full-width exp read sees initialized rows (PE-written, so
            # psum pending-zero tracking stays consistent).
            psc = p_sccx.tile([128, 512], F32, tag="sccx", name="sccx")
            s_zero = singles.tile([128, 512], BF16, tag="zz", name="zz")
            nc.vector.memset(s_zero, 0.0)
            nc.tensor.matmul(psc, sb_id16, s_zero, start=True, stop=True)
            pgT = psc[:, 448:512]

            # ---- phase 2 pump machinery ----
            ph2_items = [(mi, ni) for mi in range(len(m_chunks))
                         for ni in range(len(n_chunks))]
            ph2_state = {"pos": 0, "alt": 0, "fw_ni": None, "fw": None}
            fcw_view = d_fcW2[:, :].rearrange("p (k v) -> p k v", k=HC)

            ph2_fw = {}
            ph2_q = {"rr": False}

            def ph2_load_fw(ni):
                if ni in ph2_fw:
                    return
                no, nn = n_chunks[ni]
                fwt = trans.tile([128, HC * 512], BF16, tag="fw", name="fwt")
                nc.sync.dma_start(
                    out=fwt[:, 0:HC * nn].rearrange("p (k v) -> p k v", k=HC),
                    in_=fcw_view[:, :, no:no + nn])
                ph2_fw[ni] = fwt
                while len(ph2_fw) > 4:   # match trans pool bufs
                    ph2_fw.pop(next(iter(ph2_fw)))

            def ph2_prefetch(seq, pos):
                seen = []
                for j in range(pos, len(seq)):
                    ni = seq[j][1]
                    if ni not in seen:
                        if len(seen) >= 3:
                            break
                        seen.append(ni)
                        ph2_load_fw(ni)

            def ph2_issue(mi, ni, pot_tag="ph2"):
                no, nn = n_chunks[ni]
                mo, mp = m_chunks[mi]
                ph2_load_fw(ni)
                fwt = ph2_fw[ni]
                if pot_tag == "ph2":
                    pot = p_ph2.tile([128, 512], F32, tag="ph2", name="ph2")
                else:
                    pot = p_g.tile([128, 512], F32, tag=pot_tag, name=pot_tag)
                for k in range(HC):
                    nc.tensor.matmul(
                        pot[0:mp, 0:nn],
                        sb_h[k][:, BL + mo:BL + mo + mp],
                        fwt[:, k * nn:(k + 1) * nn],
                        start=(k == 0), stop=(k == HC - 1 and not with_fcb))
                if with_fcb:
                    nc.tensor.matmul(
                        pot[0:mp, 0:nn], ones_sb[:, 0:mp],
                        fcb_sb[:, no:no + nn],
                        start=False, stop=True, skip_group_check=True)
                obt = ob.tile([128, 512], BF16, tag="ob")
                ph2_state["alt"] ^= 1
                if ph2_state["alt"]:
                    nc.vector.tensor_copy(obt[0:mp, 0:nn], pot[0:mp, 0:nn])
                else:
                    nc.scalar.copy(obt[0:mp, 0:nn], pot[0:mp, 0:nn])
                nc.gpsimd.dma_start(out=d_out[mi, ni, 0:mp, 0:nn],
                                    in_=obt[0:mp, 0:nn])

            def ph2_pump_ready(t_done):
                # issue one tile only if its weights are already in SBUF
                if ph2_state["pos"] >= len(ph2_items):
                    return
                mi, ni = ph2_items[ph2_state["pos"]]
                mo, mp = m_chunks[mi]
                if t_done < (mo + mp - 1) // BL or ni not in ph2_fw:
                    return
                ph2_issue(mi, ni)
                ph2_state["pos"] += 1

            def ph2_pump(t_done, max_issue):
                issued = 0
                while ph2_state["pos"] < len(ph2_items) and issued < max_issue:
                    mi, ni = ph2_items[ph2_state["pos"]]
                    mo, mp = m_chunks[mi]
                    need_t = (mo + mp - 1) // BL
                    if t_done < need_t:
                        break
                    ph2_prefetch(ph2_items, ph2_state["pos"])
                    ph2_issue(mi, ni)
                    ph2_state["pos"] += 1
                    issued += 1

            # ---- phase 1: recurrence ----
            sb_c = stc.tile([128, 4 * BL], F32, tag="cst")
            nc.sync.dma_start(out=sb_c, in_=d_c0t[:, :])

            for t in range(n_steps):
                hcol = slice(t * BL, (t + 1) * BL)
                # dec_proj.T [A-part, (a,b)] weight-stationary; per-a
                # scalar-adds (psum scalar direct) + tanh pipeline so tanh_0
                # starts asap.
                pdT = p_small.tile([128, 4 * BL], F32, tag="psm")
                s_dT = st.tile([128, 4 * BL], F32, tag="dT")
                for a in range(ACk):
                    for k in range(HC):
                        nc.tensor.matmul(
                            pdT[:, a * BL:(a + 1) * BL],
                            sb_Wdec[k][:, a * 128:(a + 1) * 128],
                            sb_h[k][:, hcol],
                            start=(k == 0), stop=(k == HC - 1))
                    nc.vector.tensor_copy(s_dT[:, a * BL:(a + 1) * BL],
                                          pdT[:, a * BL:(a + 1) * BL])
                    for b in range(BL):
                        col = (a * BL + b) * S
                        nc.vector.tensor_scalar_add(
                            s_arg[:, col:col + S],
                            sb_encp[:, col:col + S],
                            s_dT[:, a * BL + b:a * BL + b + 1])
                    nc.scalar.activation(
                        s_arg[:, a * BL * S:(a + 1) * BL * S],
                        s_arg[:, a * BL * S:(a + 1) * BL * S], AF.Tanh)

                # gates: h part early (overlaps the tanh below)
                pg = [p_g.tile([BL, 512], F32, tag=f"pg{gq}", name=f"pg{gq}") for gq in range(4)]
                for gq in range(4):
                    for k in range(HC):
                        nc.tensor.matmul(
                            pg[gq],
                            sb_h[k][:, hcol],
                            sb_WgT[HC + k][:, gq * 512:(gq + 1) * 512],
                            start=(k == 0), stop=False, skip_group_check=True)

                # scores (4-way col-packed into rows {0,32,64,96})
                for a in range(ACk):
                    for b in range(BL):
                        col = (a * BL + b) * S
                        nc.tensor.matmul(
                            psc[32 * b:32 * b + 1, 0:S],
                            sb_v[:, a:a + 1],
                            s_arg[:, col:col + S],
                            start=(a == 0), stop=(a == ACk - 1),
                            tile_position=(0, 32 * b), skip_group_check=True)

                s_exp = st.tile([128, S], F32, tag="exp")
                s_sum = st.tile([128, 1], F32, tag="sum")
                s_inv = st.tile([128, 1], F32, tag="inv")
                nc.scalar.activation(s_exp, psc[:, 0:S], AF.Exp, accum_out=s_sum)
                nc.vector.reciprocal(s_inv, s_sum)

                # transpose raw exp chunks immediately (normalization deferred
                # to the ctx.T copy): alpha.T -> [S-part, 128], b at col 32b
                s_aT = st.tile([128, 4 * 128], BF16, tag="aT")
                for ci, (so, sp) in enumerate(S_CHUNKS):
                    paT = p_small.tile([128, 128], F32, tag="psm")
                    nc.tensor.transpose(
                        paT[0:sp, :], s_exp[:, so:so + sp], sb_id)
                    if ci % 2 == 0:
                        nc.vector.tensor_copy(s_aT[0:sp, ci * 128:(ci + 1) * 128],
                                              paT[0:sp, :])
                    else:
                        nc.scalar.copy(s_aT[0:sp, ci * 128:(ci + 1) * 128],
                                       paT[0:sp, :])

                # invcols[p, k*4+b] = 1/Z_b on every partition, built off the
                # chain: matmul of broadcast(1/Z) against the selection matrix
                picol = p_small.tile([128, 4 * BL], F32, tag="psm")
                nc.tensor.matmul(picol, s_inv.broadcast_to([128, 128]), sb_sel,
                                 start=True, stop=True)
                s_icol = st.tile([128, 4 * BL], F32, tag="icol")
                nc.vector.tensor_copy(s_icol, picol)

                # preload emb-gates (+biases) into the gates.T psum region.
                # Must come after the scores matmuls: start=True marks the
                # whole shared bank pending-zero on its partitions.
                nc.tensor.matmul(
                    pgT, sb_id16, sb_embb[:, t * 4 * GC:(t + 1) * 4 * GC],
                    start=True, stop=False, skip_group_check=True)

                # fused context.T: pcT[:, k*BL+b] = enc_b[k-chunk].T @ alpha_b.
                # ci innermost (a column's accumulation group must finish
                # before the next column's start=True re-marks the bank);
                # k outermost so gates-ctx for k can start early.
                pcT = p_small.tile([128, 4 * BL], F32, tag="psm")
                s_cT = st.tile([128, 4 * BL], BF16, tag="cT")
                s_g = stg.tile([BL, G], F32, tag="g")
                for k in range(HC):
                    for b in range(BL):
                        col = k * BL + b
                        for ci, (so, sp) in enumerate(S_CHUNKS):
                            nc.tensor.matmul(
                                pcT[:, col:col + 1],
                                sb_enc[b][ci][0:sp, k * 128:(k + 1) * 128],
                                s_aT[0:sp, ci * 128 + 32 * b:ci * 128 + 32 * b + 1],
                                start=(ci == 0), stop=(ci == len(S_CHUNKS) - 1),
                                skip_group_check=True)
                    nc.vector.tensor_mul(s_cT[:, k * BL:(k + 1) * BL],
                                         pcT[:, k * BL:(k + 1) * BL],
                                         s_icol[:, k * BL:(k + 1) * BL])

                # gates ctx part (finishes accumulation); per-gq psum->sbuf
                # copy + transpose right after each quadrant's last MM.
                # Quadrant order i,f,o,g: after quadrant 2, sigmoid(i,f,o) and
                # u1=sigma_f*c run while PE finishes quadrant 3 (g); only
                # tanh(g) -> u2 -> c -> tanh(c) -> h sit on the chain tail.
                s_act = st.tile([128, 4 * GC], F32, tag="gact")
                s_sig = st.tile([128, 48], F32, tag="sig")
                u1 = st.tile([128, 16], F32, tag="u1")
                u2 = st.tile([128, 16], F32, tag="u2")
                for gq in range(4):
                    for k in range(HC):
                        nc.tensor.matmul(
                            pg[gq],
                            s_cT[:, k * BL:(k + 1) * BL],
                            sb_WgT[k][:, gq * 512:(gq + 1) * 512],
                            start=False, stop=(k == HC - 1), skip_group_check=True)
                    dst = s_g[:, gq * 512:(gq + 1) * 512]
                    if gq % 2 == 0:
                        nc.scalar.copy(dst, pg[gq])
                    else:
                        nc.vector.tensor_copy(dst, pg[gq])
                    for gc in range(gq * 4, (gq + 1) * 4):
                        nc.tensor.matmul(
                            pgT[:, gc * BL:(gc + 1) * BL],
                            s_g[0:BL, gc * 128:(gc + 1) * 128],
                            sb_id[0:BL, 0:BL],
                            is_transpose=True, start=False,
                            stop=(gc == GC - 1),
                            skip_group_check=True)
                    if gq == 2:
                        # i 0:16 | f 16:32 | o 32:48 ready
                        nc.scalar.activation(s_act[:, 0:48], pgT[:, 0:48],
                                             AF.Tanh, scale=0.5)
                        nc.vector.tensor_scalar(
                            s_sig, s_act[:, 0:48], 1.0, 0.5,
                            mybir.AluOpType.add, mybir.AluOpType.mult)
                        nc.vector.tensor_mul(u1, s_sig[:, 16:32], sb_c)

                nc.scalar.activation(s_act[:, 48:64], pgT[:, 48:64], AF.Tanh)
                si, so_ = s_sig[:, 0:16], s_sig[:, 32:48]
                tg = s_act[:, 48:64]
                nc.vector.tensor_mul(u2, si, tg)
                c_new = stc.tile([128, 4 * BL], F32, tag="cst")
                nc.vector.tensor_add(c_new, u1, u2)
                sb_c = c_new
                tc_t = st.tile([128, 16], F32, tag="tc")
                nc.scalar.activation(tc_t, c_new, AF.Tanh)
                for k in range(HC):
                    nc.vector.tensor_mul(
                        sb_h[k][:, (t + 1) * BL:(t + 2) * BL],
                        so_[:, k * BL:(k + 1) * BL],
                        tc_t[:, k * BL:(k + 1) * BL])

                if DEBUG and t == 0:
                    nc.gpsimd.dma_start(out=d_dbg["dT"][:, :], in_=s_dT)
                    nc.gpsimd.dma_start(out=d_dbg["alpha"][:, :], in_=s_exp)
                    nc.gpsimd.dma_start(out=d_dbg["cT"][:, :], in_=s_cT)
                    nc.gpsimd.dma_start(out=d_dbg["act"][:, :], in_=s_act)
                    nc.gpsimd.dma_start(out=d_dbg["arg"][:, :], in_=s_arg)

                # pump phase-2 logit tiles whose h block is complete
                ph2_pump(t, PH2_PER_STEP)

            # ---- phase 2 remainder: ni-major so one weight DMA serves all
            # remaining m-chunks of that n-chunk ----
            rest = ph2_items[ph2_state["pos"]:]
            rest.sort(key=lambda it: (it[1], it[0]))
            tags = ["ph2", "pg0", "pg1", "pg2"]
            ph2_q["rr"] = True   # spread tail weight loads over 4 DMA queues
            for i, (mi, ni) in enumerate(rest):
                ph2_prefetch(rest, i)
                ph2_issue(mi, ni, pot_tag=tags[i % 4])
    nc.finalize()
    return nc


def _prep_inputs(encoder_outputs, hidden0, cell0, summary, summary_len,
                 embedding, W_ih, b_ih, W_hh, b_hh,
                 att_Wenc, att_benc, att_Wdec, att_bdec, att_v,
                 fc_W, fc_b):
    f32 = np.float32
    bf16 = ml_dtypes.bfloat16
    summary = np.asarray(summary)
    sm = summary.copy()
    sm[np.arange(B), np.asarray(summary_len) - 1] = 0
    sm = sm[:, :-1]                                  # [B, 99]
    emb = np.asarray(embedding, f32)[sm]             # [B, 99, E]

    # gate reorder i,f,g,o -> i,f,o,g
    r = np.arange(G)
    perm = np.concatenate([r[0:H], r[H:2 * H], r[3 * H:4 * H], r[2 * H:3 * H]])
    W_ih_p = np.asarray(W_ih, f32)[perm]
    W_hh_p = np.asarray(W_hh, f32)[perm]
    bg_p = (np.asarray(b_ih, f32) + np.asarray(b_hh, f32))[perm]

    # emb-part of gates for all steps (+ gate biases), on host
    eg = np.einsum('bte,ge->btg', emb, W_ih_p[:, :E]) + bg_p   # [B,99,G]

    WgT = np.ascontiguousarray(np.concatenate(
        [W_ih_p[:, E:].T, W_hh_p.T], axis=0))        # [2H, G]
    vt = np.ascontiguousarray(np.asarray(att_v, f32).reshape(4, 128).T).astype(bf16)
    ident = np.eye(128, dtype=f32)
    # fcW2[p, k*V + v] = fc_W[v, k*128 + p]
    fcW2 = np.ascontiguousarray(
        np.asarray(fc_W, f32).T.reshape(HC, 128, V).transpose(1, 0, 2)
        .reshape(128, HC * V)).astype(bf16)
    fcb = np.ascontiguousarray(np.asarray(fc_b, f32).reshape(1, V)).astype(bf16)
    with_fcb = bool(np.any(np.asarray(fc_b)))

    h0 = np.asarray(hidden0, f32)
    c0 = np.asarray(cell0, f32)
    enc = np.asarray(encoder_outputs, f32)
    bde = np.asarray(att_benc, f32) + np.asarray(att_bdec, f32)

    WgT16 = WgT.astype(bf16)
    Wdec16 = np.ascontiguousarray(np.asarray(att_Wdec, f32)).astype(bf16)
    in_maps = []
    for c in range(NC):
        bs = slice(c * BL, (c + 1) * BL)
        h0T = np.ascontiguousarray(h0[bs].T).astype(bf16)
        c0t = np.ascontiguousarray(
            c0[bs].T.reshape(4, 128, BL).transpose(1, 0, 2).reshape(128, 4 * BL))
        # enc_proj (+benc+bdec), transposed: [128, a*1600 + b*400 + s]
        encp = enc[bs] @ np.asarray(att_Wenc, f32) + bde      # [BL, S, A]
        encp_dev = np.ascontiguousarray(
            encp.transpose(2, 0, 1).reshape(ACk, 128, BL, S)
            .transpose(1, 0, 2, 3).reshape(128, ACk * BL * S)).astype(bf16)
        # emb-gates: [128, t*64 + gc*4 + b]
        embb = np.ascontiguousarray(
            eg[bs].reshape(BL, TS, GC, 128).transpose(3, 1, 2, 0)
            .reshape(128, TS * 4 * GC)).astype(bf16)
        sel = np.zeros((128, 16), np.float32)
        for k in range(HC):
            for b in range(BL):
                sel[32 * b, k * 4 + b] = 1.0
        in_maps.append({
            "enc": np.ascontiguousarray(enc[bs]).astype(bf16),
            "sel": sel,
            "encp": encp_dev,
            "h0T": h0T, "c0t": c0t, "embb": embb,
            "WgT": WgT16,
            "Wdec": Wdec16,
            "vt": vt,
            "ident": ident, "id16": ident.astype(bf16),
            "fcW2": fcW2, "fcb": fcb,
        })
    return in_maps, with_fcb


_NC_CACHE = {}


def kernel(**inputs):
    in_maps, with_fcb = _prep_inputs(**inputs)
    key = (TS, with_fcb)
    if key not in _NC_CACHE:
        _NC_CACHE[key] = build_nc(TS, with_fcb)
    nc = _NC_CACHE[key]
    res = run_bass_kernel_spmd(nc, in_maps, list(range(NC)))
    ts_rows = TS * BL
    n_m = ceil_div(ts_rows, 128)
    outs = []
    for c in range(NC):
        o4 = np.asarray(res.results[c]["out"])     # [n_m, n_n, 128, 512]
        n_n = o4.shape[1]
        rows = []
        for mi in range(n_m):
            mp = min(128, ts_rows - mi * 128)
            blk = o4[mi, :, 0:mp, :]               # [n_n, mp, 512]
            rows.append(blk.transpose(1, 0, 2).reshape(mp, n_n * 512)[:, :V])
        o = np.concatenate(rows, axis=0).astype(np.float32)  # [(t,b), V]
        outs.append(o.reshape(TS, BL, V).transpose(1, 0, 2))
    return np.concatenate(outs, axis=0)


# revision 72
# speedup vs baseline: 1.0027x; 1.0027x over previous
"""DecoderLSTM w/ Bahdanau attention for trn2, 8 NeuronCores, data-parallel over batch.

Sharding: batch B=32 -> 4 samples/core, zero collectives.
Host precomputes: enc_proj (+benc+bdec) transposed per a-chunk; emb-part of
LSTM gates (+biases) per step; gate reorder i,f,g,o -> i,f,o,g so
sigmoid(i,f,o) is one tanh(x/2) ACT instr; fc weights in [128, k*V+v] layout
for single-DMA n-chunk loads.
Phase 1: 99 sequential steps fully SBUF-resident:
  dec_proj.T [A-part,(a,b)] -> DVE per-partition-scalar adds (4x mode) into
  s_arg [128, (a,b,s)] -> 4 big ACT tanh [128,1600] (pipelines with scores)
  scores = v.T @ T (tile-position packed into psum rows {0,32,64,96})
  single exp+accum over [128,400]; alpha.T (raw exp) via 4 PE transposes;
  softmax 1/Z deferred: an off-chain matmul of broadcast(1/Z) against a
  selection matrix builds a per-column inverse tile applied at the ctx.T
  psum->sbuf copy. Fused context.T: 64 accumulating matvecs produce
  pcT[:, k*4+b] directly (no separate ctx + transpose stages).
  gates = h-part (early, fills the tanh window) + ctx-part weight-moving MMs;
  emb-part preloaded into psum via identity matmul, gate transposes
  accumulate onto it; activations read psum; sigmoid(i,f,o)+u1 fire after
  quadrant 2 while PE finishes g.
  Phase-2 logit tiles (h_hist @ fc_W.T, bf16 block-layout out, host
  reassembles) pumped into PE idle windows as their 32-step h block
  completes; ni-major remainder with prefetched weight loads and psum-bank
  rotation over the then-idle gate banks.
"""

import numpy as np
import ml_dtypes

import concourse.bass as bass
import concourse.bacc as bacc
import concourse.mybir as mybir
import concourse.tile as tile
from concourse.bass_utils import run_bass_kernel_spmd

B, S, T = 32, 400, 100
E, H, A, V = 256, 512, 512, 32000
G = 4 * H
NC = 8
BL = B // NC          # 4 local samples
TS = T - 1            # 99 steps
F32 = mybir.dt.float32
BF16 = mybir.dt.bfloat16
AF = mybir.ActivationFunctionType

HC = H // 128   # 4
ACk = A // 128  # 4
KC = 2 * HC     # 8 gate K chunks (ctx, h)
GC = G // 128   # 16
S_CHUNKS = [(i * 128, min(128, S - i * 128)) for i in range((S + 127) // 128)]
PH2_PER_STEP = 4


def ceil_div(a, b):
    return (a + b - 1) // b


DEBUG = False


def build_nc(n_steps=TS, with_fcb=False):
    nc = bacc.Bacc()
    ts_rows = n_steps * BL
    if DEBUG:
        d_dbg = {
            "dT": nc.dram_tensor("dbg_dT", [128, 16], F32, kind="ExternalOutput"),
            "alpha": nc.dram_tensor("dbg_alpha", [128, S], F32, kind="ExternalOutput"),
            "cT": nc.dram_tensor("dbg_cT", [128, 16], F32, kind="ExternalOutput"),
            "act": nc.dram_tensor("dbg_act", [128, 64], F32, kind="ExternalOutput"),
            "arg": nc.dram_tensor("dbg_arg", [128, 6400], F32, kind="ExternalOutput"),
        }

    d_enc = nc.dram_tensor("enc", [BL, S, H], BF16, kind="ExternalInput")
    d_encp = nc.dram_tensor("encp", [128, ACk * BL * S], BF16, kind="ExternalInput")
    d_h0T = nc.dram_tensor("h0T", [H, BL], BF16, kind="ExternalInput")
    d_c0t = nc.dram_tensor("c0t", [128, 4 * BL], F32, kind="ExternalInput")
    d_embb = nc.dram_tensor("embb", [128, n_steps * 4 * GC], BF16, kind="ExternalInput")
    d_WgT = nc.dram_tensor("WgT", [2 * H, G], BF16, kind="ExternalInput")
    d_Wdec = nc.dram_tensor("Wdec", [H, A], BF16, kind="ExternalInput")
    d_v = nc.dram_tensor("vt", [128, 4], BF16, kind="ExternalInput")
    d_ident = nc.dram_tensor("ident", [128, 128], F32, kind="ExternalInput")
    d_id16 = nc.dram_tensor("id16", [128, 128], BF16, kind="ExternalInput")
    # selection matrix: SEL[32b, k*4+b] = 1 -> invcols[p, k*4+b] = 1/Z_b
    d_sel = nc.dram_tensor("sel", [128, 4 * BL], F32, kind="ExternalInput")
    # fc weights: [128, k*V + v] so one n-chunk slice is 4 strided ranges
    # fetched in a single DMA
    d_fcW2 = nc.dram_tensor("fcW2", [128, HC * V], BF16, kind="ExternalInput")
    d_fcb = nc.dram_tensor("fcb", [1, V], BF16, kind="ExternalInput")

    n_chunks = [(i * 512, min(512, V - i * 512)) for i in range(ceil_div(V, 512))]
    m_chunks = [(i * 128, min(128, ts_rows - i * 128))
                for i in range(ceil_div(ts_rows, 128))]
    # block-layout output: one contiguous DMA per logit tile; host reassembles
    d_out = nc.dram_tensor(
        "out", [len(m_chunks), len(n_chunks), 128, 512], BF16,
        kind="ExternalOutput")

    with tile.TileContext(nc) as tc:
        import contextlib
        stack = contextlib.ExitStack()
        with stack:
            P = lambda name, bufs, space="SBUF": stack.enter_context(
                tc.tile_pool(name=name, bufs=bufs, space=space))
            singles = P("singles", 1)
            trans = P("trans", 4)    # transient sbuf (ph2 fw tiles)
            st = P("st", 2)          # small per-step sbuf scratch
            stg = P("stg", 1)        # [4, 2048] gates sbuf
            stc = P("stc", 2)        # c state ping-pong
            Tp = P("Tp", 1)          # big [128, 6400] tanh-arg tile
            ob = P("ob", 6)          # phase2 out staging
            # PSUM: exactly 8 banks
            p_small = P("p_small", 2, space="PSUM")   # tag psm, <=512B
            p_sccx = P("p_sccx", 1, space="PSUM")     # shared scores/ctx bank
            p_g = P("p_g", 1, space="PSUM")           # tags pg0..3, 2KB each
            p_ph2 = P("p_ph2", 1, space="PSUM")       # phase2 logits bank

            # ---- persistent SBUF ----
            sb_enc = [[singles.tile([sp, H], BF16, tag=f"enc_{b}_{ci}", name=f"enc_{b}_{ci}")
                       for ci, (so, sp) in enumerate(S_CHUNKS)] for b in range(BL)]
            for b in range(BL):
                for ci, (so, sp) in enumerate(S_CHUNKS):
                    nc.sync.dma_start(out=sb_enc[b][ci], in_=d_enc[b, so:so + sp, :])
            sb_encp = singles.tile([128, ACk * BL * S], BF16, tag="encp", name="encp")
            nc.sync.dma_start(out=sb_encp, in_=d_encp[:, :])
            sb_embb = singles.tile([128, n_steps * 4 * GC], BF16, tag="embb", name="embb")
            nc.sync.dma_start(out=sb_embb, in_=d_embb[:, :])
            sb_WgT = [singles.tile([128, G], BF16, tag=f"wg_{k}", name=f"wg_{k}") for k in range(KC)]
            for k in range(KC):
                nc.sync.dma_start(out=sb_WgT[k], in_=d_WgT[k * 128:(k + 1) * 128, :])
            sb_Wdec = [singles.tile([128, A], BF16, tag=f"wd_{k}", name=f"wd_{k}") for k in range(HC)]
            for k in range(HC):
                nc.sync.dma_start(out=sb_Wdec[k], in_=d_Wdec[k * 128:(k + 1) * 128, :])
            sb_v = singles.tile([128, 4], BF16)
            nc.sync.dma_start(out=sb_v, in_=d_v[:, :])
            sb_id = singles.tile([128, 128], F32)
            nc.sync.dma_start(out=sb_id, in_=d_ident[:, :])
            sb_id16 = singles.tile([128, 128], BF16)
            nc.sync.dma_start(out=sb_id16, in_=d_id16[:, :])
            sb_sel = singles.tile([128, 4 * BL], F32)
            nc.sync.dma_start(out=sb_sel, in_=d_sel[:, :])
            # h history: one tile, col = k*HB + t*BL + b (HB per k-chunk)
            HB = (TS + 1) * BL
            sb_hh = singles.tile([128, HC * HB], BF16, tag="hh", name="hh")
            sb_h = [sb_hh[:, k * HB:(k + 1) * HB] for k in range(HC)]
            for k in range(HC):
                nc.sync.dma_start(out=sb_hh[:, k * HB:k * HB + BL],
                                  in_=d_h0T[k * 128:(k + 1) * 128, :])
            if with_fcb:
                ones_sb = singles.tile([1, 128], BF16)
                nc.vector.memset(ones_sb, 1.0)
                fcb_sb = singles.tile([1, V], BF16)
                nc.sync.dma_start(out=fcb_sb, in_=d_fcb[:, :])

            # tanh-argument tile, written by DVE scalar-adds, tanh in-place
            s_arg = Tp.tile([128, ACk * BL * S], BF16, tag="targ", name="targ")

            # persistent psum bank: scores in cols 0:400 (rows 32b), gates.T
            # (+emb preload) in cols 448:512. Zero-init via matmul once so the
            # single full-width exp read sees initialized rows (PE-written, so
            # psum pending-zero tracking stays consistent).
            psc = p_sccx.tile([128, 512], F32, tag="sccx", name="sccx")
            s_zero = singles.tile([128, 512], BF16, tag="zz", name="zz")
            nc.vector.memset(s_zero, 0.0)
            nc.tensor.matmul(psc, sb_id16, s_zero, start=True, stop=True)
            pgT = psc[:, 448:512]

            # ---- phase 2 pump machinery ----
            ph2_items = [(mi, ni) for mi in range(len(m_chunks))
                         for ni in range(len(n_chunks))]
            ph2_state = {"pos": 0, "alt": 0, "fw_ni": None, "fw": None}
            fcw_view = d_fcW2[:, :].rearrange("p (k v) -> p k v", k=HC)

            ph2_fw = {}
            ph2_q = {"rr": False}

            def ph2_load_fw(ni):
                if ni in ph2_fw:
                    return
                no, nn = n_chunks[ni]
                fwt = trans.tile([128, HC * 512], BF16, tag="fw", name="fwt")
                nc.sync.dma_start(
                    out=fwt[:, 0:HC * nn].rearrange("p (k v) -> p k v", k=HC),
                    in_=fcw_view[:, :, no:no + nn])
                ph2_fw[ni] = fwt
                while len(ph2_fw) > 4:   # match trans pool bufs
                    ph2_fw.pop(next(iter(ph2_fw)))

            def ph2_prefetch(seq, pos):
                seen = []
                for j in range(pos, len(seq)):
                    ni = seq[j][1]
                    if ni not in seen:
                        if len(seen) >= 3:
                            break
                        seen.append(ni)
                        ph2_load_fw(ni)

            def ph2_issue(mi, ni, pot_tag="ph2"):
                no, nn = n_chunks[ni]
                mo, mp = m_chunks[mi]
                ph2_load_fw(ni)
                fwt = ph2_fw[ni]
                if pot_tag == "ph2":
                    pot = p_ph2.tile([128, 512], F32, tag="ph2", name="ph2")
                else:
                    pot = p_g.tile([128, 512], F32, tag=pot_tag, name=pot_tag)
                for k in range(HC):
                    nc.tensor.matmul(
                        pot[0:mp, 0:nn],
                        sb_h[k][:, BL + mo:BL + mo + mp],
                        fwt[:, k * nn:(k + 1) * nn],
                        start=(k == 0), stop=(k == HC - 1 and not with_fcb))
                if with_fcb:
                    nc.tensor.matmul(
                        pot[0:mp, 0:nn], ones_sb[:, 0:mp],
                        fcb_sb[:, no:no + nn],
                        start=False, stop=True, skip_group_check=True)
                obt = ob.tile([128, 512], BF16, tag="ob")
                ph2_state["alt"] ^= 1
                if ph2_state["alt"]:
                    nc.vector.tensor_copy(obt[0:mp, 0:nn], pot[0:mp, 0:nn])
                else:
                    nc.scalar.copy(obt[0:mp, 0:nn], pot[0:mp, 0:nn])
                nc.gpsimd.dma_start(out=d_out[mi, ni, 0:mp, 0:nn],
                                    in_=obt[0:mp, 0:nn])

            def ph2_pump_ready(t_done):
                # issue one tile only if its weights are already in SBUF
                if ph2_state["pos"] >= len(ph2_items):
                    return
                mi, ni = ph2_items[ph2_state["pos"]]
                mo, mp = m_chunks[mi]
                if t_done < (mo + mp - 1) // BL or ni not in ph2_fw:
                    return
                ph2_issue(mi, ni)
                ph2_state["pos"] += 1

            def ph2_pump(t_done, max_issue):
                issued = 0
                while ph2_state["pos"] < len(ph2_items) and issued < max_issue:
                    mi, ni = ph2_items[ph2_state["pos"]]
                    mo, mp = m_chunks[mi]
                    need_t = (mo + mp - 1) // BL
                    if t_done < need_t:
                        break
                    ph2_prefetch(ph2_items, ph2_state["pos"])
                    ph2_issue(mi, ni)
                    ph2_state["pos"] += 1
                    issued += 1

            # ---- phase 1: recurrence ----
            sb_c = stc.tile([128, 4 * BL], F32, tag="cst")
            nc.sync.dma_start(out=sb_c, in_=d_c0t[:, :])

            for t in range(n_steps):
                hcol = slice(t * BL, (t + 1) * BL)
                # dec_proj.T [A-part, (a,b)] weight-stationary; per-a
                # scalar-adds (psum scalar direct) + tanh pipeline so tanh_0
                # starts asap.
                pdT = p_small.tile([128, 4 * BL], F32, tag="psm")
                s_dT = st.tile([128, 4 * BL], F32, tag="dT")
                for a in range(ACk):
                    for k in range(HC):
                        nc.tensor.matmul(
                            pdT[:, a * BL:(a + 1) * BL],
                            sb_Wdec[k][:, a * 128:(a + 1) * 128],
                            sb_h[k][:, hcol],
                            start=(k == 0), stop=(k == HC - 1))
                    nc.vector.tensor_copy(s_dT[:, a * BL:(a + 1) * BL],
                                          pdT[:, a * BL:(a + 1) * BL])
                    for b in range(BL):
                        col = (a * BL + b) * S
                        nc.vector.tensor_scalar_add(
                            s_arg[:, col:col + S],
                            sb_encp[:, col:col + S],
                            s_dT[:, a * BL + b:a * BL + b + 1])
                    nc.scalar.activation(
                        s_arg[:, a * BL * S:(a + 1) * BL * S],
                        s_arg[:, a * BL * S:(a + 1) * BL * S], AF.Tanh)

                # gates: h part early (overlaps the tanh below)
                pg = [p_g.tile([BL, 512], F32, tag=f"pg{gq}", name=f"pg{gq}") for gq in range(4)]
                for gq in range(4):
                    for k in range(HC):
                        nc.tensor.matmul(
                            pg[gq],
                            sb_h[k][:, hcol],
                            sb_WgT[HC + k][:, gq * 512:(gq + 1) * 512],
                            start=(k == 0), stop=False, skip_group_check=True)

                # scores (4-way col-packed into rows {0,32,64,96})
                for a in range(ACk):
                    for b in range(BL):
                        col = (a * BL + b) * S
                        nc.tensor.matmul(
                            psc[32 * b:32 * b + 1, 0:S],
                            sb_v[:, a:a + 1],
                            s_arg[:, col:col + S],
                            start=(a == 0), stop=(a == ACk - 1),
                            tile_position=(0, 32 * b), skip_group_check=True)

                s_exp = st.tile([128, S], F32, tag="exp")
                s_sum = st.tile([128, 1], F32, tag="sum")
                s_inv = st.tile([128, 1], F32, tag="inv")
                nc.scalar.activation(s_exp, psc[:, 0:S], AF.Exp, accum_out=s_sum)
                nc.vector.reciprocal(s_inv, s_sum)

                # transpose raw exp chunks immediately (normalization deferred
                # to the ctx.T copy): alpha.T -> [S-part, 128], b at col 32b
                s_aT = st.tile([128, 4 * 128], BF16, tag="aT")
                for ci, (so, sp) in enumerate(S_CHUNKS):
                    paT = p_small.tile([128, 128], F32, tag="psm")
                    nc.tensor.transpose(
                        paT[0:sp, :], s_exp[:, so:so + sp], sb_id)
                    if ci % 2 == 0:
                        nc.vector.tensor_copy(s_aT[0:sp, ci * 128:(ci + 1) * 128],
                                              paT[0:sp, :])
                    else:
                        nc.scalar.copy(s_aT[0:sp, ci * 128:(ci + 1) * 128],
                                       paT[0:sp, :])

                # invcols[p, k*4+b] = 1/Z_b on every partition, built off the
                # chain: matmul of broadcast(1/Z) against the selection matrix
                picol = psc[:, 400:400 + 4 * BL]
                nc.tensor.matmul(picol, s_inv.broadcast_to([128, 128]), sb_sel,
                                 start=True, stop=True, skip_group_check=True)
                s_icol = st.tile([128, 4 * BL], F32, tag="icol")
                nc.vector.tensor_copy(s_icol, picol)

                # preload emb-gates (+biases) into the gates.T psum region.
                # Must come after the scores matmuls: start=True marks the
                # whole shared bank pending-zero on its partitions.
                nc.tensor.matmul(
                    pgT, sb_id16, sb_embb[:, t * 4 * GC:(t + 1) * 4 * GC],
                    start=True, stop=False, skip_group_check=True)

                # fused context.T: pcT[:, k*BL+b] = enc_b[k-chunk].T @ alpha_b.
                # ci innermost (a column's accumulation group must finish
                # before the next column's start=True re-marks the bank);
                # k outermost so gates-ctx for k can start early.
                pcT = p_small.tile([128, 4 * BL], F32, tag="psm")
                s_cT = st.tile([128, 4 * BL], BF16, tag="cT")
                s_g = stg.tile([BL, G], F32, tag="g")
                for k in range(HC):
                    for b in range(BL):
                        col = k * BL + b
                        for ci, (so, sp) in enumerate(S_CHUNKS):
                            nc.tensor.matmul(
                                pcT[:, col:col + 1],
                                sb_enc[b][ci][0:sp, k * 128:(k + 1) * 128],
                                s_aT[0:sp, ci * 128 + 32 * b:ci * 128 + 32 * b + 1],
                                start=(ci == 0), stop=(ci == len(S_CHUNKS) - 1),
                                skip_group_check=True)
                    nc.vector.tensor_mul(s_cT[:, k * BL:(k + 1) * BL],
                                         pcT[:, k * BL:(k + 1) * BL],
                                         s_icol[:, k * BL:(k + 1) * BL])

                # gates ctx part (finishes accumulation); per-gq psum->sbuf
                # copy + transpose right after each quadrant's last MM.
                # Quadrant order i,f,o,g: after quadrant 2, sigmoid(i,f,o) and
                # u1=sigma_f*c run while PE finishes quadrant 3 (g); only
                # tanh(g) -> u2 -> c -> tanh(c) -> h sit on the chain tail.
                s_act = st.tile([128, 4 * GC], F32, tag="gact")
                s_sig = st.tile([128, 48], F32, tag="sig")
                u1 = st.tile([128, 16], F32, tag="u1")
                u2 = st.tile([128, 16], F32, tag="u2")
                for gq in range(4):
                    for k in range(HC):
                        nc.tensor.matmul(
                            pg[gq],
                            s_cT[:, k * BL:(k + 1) * BL],
                            sb_WgT[k][:, gq * 512:(gq + 1) * 512],
                            start=False, stop=(k == HC - 1), skip_group_check=True)
                    dst = s_g[:, gq * 512:(gq + 1) * 512]
                    if gq % 2 == 0:
                        nc.scalar.copy(dst, pg[gq])
                    else:
                        nc.vector.tensor_copy(dst, pg[gq])
                    for gc in range(gq * 4, (gq + 1) * 4):
                        nc.tensor.matmul(
                            pgT[:, gc * BL:(gc + 1) * BL],
                            s_g[0:BL, gc * 128:(gc + 1) * 128],
                            sb_id[0:BL, 0:BL],
                            is_transpose=True, start=False,
                            stop=(gc == GC - 1),
                            skip_group_check=True)
                    if gq == 2:
                        # i 0:16 | f 16:32 | o 32:48 ready
                        nc.scalar.activation(s_act[:, 0:48], pgT[:, 0:48],
                                             AF.Tanh, scale=0.5)
                        nc.vector.tensor_scalar(
                            s_sig, s_act[:, 0:48], 1.0, 0.5,
                            mybir.AluOpType.add, mybir.AluOpType.mult)
                        nc.vector.tensor_mul(u1, s_sig[:, 16:32], sb_c)

                nc.scalar.activation(s_act[:, 48:64], pgT[:, 48:64], AF.Tanh)
                si, so_ = s_sig[:, 0:16], s_sig[:, 32:48]
                tg = s_act[:, 48:64]
                nc.vector.tensor_mul(u2, si, tg)
                c_new = stc.tile([128, 4 * BL], F32, tag="cst")
                nc.vector.tensor_add(c_new, u1, u2)
                sb_c = c_new
                tc_t = st.tile([128, 16], F32, tag="tc")
                nc.scalar.activation(tc_t, c_new, AF.Tanh)
                for k in range(HC):
                    nc.vector.tensor_mul(
                        sb_h[k][:, (t + 1) * BL:(t + 2) * BL],
                        so_[:, k * BL:(k + 1) * BL],
                        tc_t[:, k * BL:(k + 1) * BL])

                if DEBUG and t == 0:
                    nc.gpsimd.dma_start(out=d_dbg["dT"][:, :], in_=s_dT)
                    nc.gpsimd.dma_start(out=d_dbg["alpha"][:, :], in_=s_exp)
                    nc.gpsimd.dma_start(out=d_dbg["cT"][:, :], in_=s_cT)
                    nc.gpsimd.dma_start(out=d_dbg["act"][:, :], in_=s_act)
                    nc.gpsimd.dma_start(out=d_dbg["arg"][:, :], in_=s_arg)

                # pump phase-2 logit tiles whose h block is complete
                ph2_pump(t, PH2_PER_STEP)

            # ---- phase 2 remainder: ni-major so one weight DMA serves all
            # remaining m-chunks of that n-chunk ----
            rest = ph2_items[ph2_state["pos"]:]
            rest.sort(key=lambda it: (it[1], it[0]))
            tags = ["ph2", "pg0", "pg1", "pg2"]
            ph2_q["rr"] = True   # spread tail weight loads over 4 DMA queues
            for i, (mi, ni) in enumerate(rest):
                ph2_prefetch(rest, i)
                ph2_issue(mi, ni, pot_tag=tags[i % 4])
    nc.finalize()
    return nc


def _prep_inputs(encoder_outputs, hidden0, cell0, summary, summary_len,
                 embedding, W_ih, b_ih, W_hh, b_hh,
                 att_Wenc, att_benc, att_Wdec, att_bdec, att_v,
                 fc_W, fc_b):
    f32 = np.float32
    bf16 = ml_dtypes.bfloat16
    summary = np.asarray(summary)
    sm = summary.copy()
    sm[np.arange(B), np.asarray(summary_len) - 1] = 0
    sm = sm[:, :-1]                                  # [B, 99]
    emb = np.asarray(embedding, f32)[sm]             # [B, 99, E]

    # gate reorder i,f,g,o -> i,f,o,g
    r = np.arange(G)
    perm = np.concatenate([r[0:H], r[H:2 * H], r[3 * H:4 * H], r[2 * H:3 * H]])
    W_ih_p = np.asarray(W_ih, f32)[perm]
    W_hh_p = np.asarray(W_hh, f32)[perm]
    bg_p = (np.asarray(b_ih, f32) + np.asarray(b_hh, f32))[perm]

    # emb-part of gates for all steps (+ gate biases), on host
    eg = np.einsum('bte,ge->btg', emb, W_ih_p[:, :E]) + bg_p   # [B,99,G]

    WgT = np.ascontiguousarray(np.concatenate(
        [W_ih_p[:, E:].T, W_hh_p.T], axis=0))        # [2H, G]
    vt = np.ascontiguousarray(np.asarray(att_v, f32).reshape(4, 128).T).astype(bf16)
    ident = np.eye(128, dtype=f32)
    # fcW2[p, k*V + v] = fc_W[v, k*128 + p]
    fcW2 = np.ascontiguousarray(
        np.asarray(fc_W, f32).T.reshape(HC, 128, V).transpose(1, 0, 2)
        .reshape(128, HC * V)).astype(bf16)
    fcb = np.ascontiguousarray(np.asarray(fc_b, f32).reshape(1, V)).astype(bf16)
    with_fcb = bool(np.any(np.asarray(fc_b)))

    h0 = np.asarray(hidden0, f32)
    c0 = np.asarray(cell0, f32)
    enc = np.asarray(encoder_outputs, f32)
    bde = np.asarray(att_benc, f32) + np.asarray(att_bdec, f32)

    WgT16 = WgT.astype(bf16)
    Wdec16 = np.ascontiguousarray(np.asarray(att_Wdec, f32)).astype(bf16)
    in_maps = []
    for c in range(NC):
        bs = slice(c * BL, (c + 1) * BL)
        h0T = np.ascontiguousarray(h0[bs].T).astype(bf16)
        c0t = np.ascontiguousarray(
            c0[bs].T.reshape(4, 128, BL).transpose(1, 0, 2).reshape(128, 4 * BL))
        # enc_proj (+benc+bdec), transposed: [128, a*1600 + b*400 + s]
        encp = enc[bs] @ np.asarray(att_Wenc, f32) + bde      # [BL, S, A]
        encp_dev = np.ascontiguousarray(
            encp.transpose(2, 0, 1).reshape(ACk, 128, BL, S)
            .transpose(1, 0, 2, 3).reshape(128, ACk * BL * S)).astype(bf16)
        # emb-gates: [128, t*64 + gc*4 + b]
        embb = np.ascontiguousarray(
            eg[bs].reshape(BL, TS, GC, 128).transpose(3, 1, 2, 0)
            .reshape(128, TS * 4 * GC)).astype(bf16)
        sel = np.zeros((128, 16), np.float32)
        for k in range(HC):
            for b in range(BL):
                sel[32 * b, k * 4 + b] = 1.0
        in_maps.append({
            "enc": np.ascontiguousarray(enc[bs]).astype(bf16),
            "sel": sel,
            "encp": encp_dev,
            "h0T": h0T, "c0t": c0t, "embb": embb,
            "WgT": WgT16,
            "Wdec": Wdec16,
            "vt": vt,
            "ident": ident, "id16": ident.astype(bf16),
            "fcW2": fcW2, "fcb": fcb,
        })
    return in_maps, with_fcb


_NC_CACHE = {}


def kernel(**inputs):
    in_maps, with_fcb = _prep_inputs(**inputs)
    key = (TS, with_fcb)
    if key not in _NC_CACHE:
        _NC_CACHE[key] = build_nc(TS, with_fcb)
    nc = _NC_CACHE[key]
    res = run_bass_kernel_spmd(nc, in_maps, list(range(NC)))
    ts_rows = TS * BL
    n_m = ceil_div(ts_rows, 128)
    outs = []
    for c in range(NC):
        o4 = np.asarray(res.results[c]["out"])     # [n_m, n_n, 128, 512]
        n_n = o4.shape[1]
        rows = []
        for mi in range(n_m):
            mp = min(128, ts_rows - mi * 128)
            blk = o4[mi, :, 0:mp, :]               # [n_n, mp, 512]
            rows.append(blk.transpose(1, 0, 2).reshape(mp, n_n * 512)[:, :V])
        o = np.concatenate(rows, axis=0).astype(np.float32)  # [(t,b), V]
        outs.append(o.reshape(TS, BL, V).transpose(1, 0, 2))
    return np.concatenate(outs, axis=0)
